# revision 20
# baseline (speedup 1.0000x reference)
"""Trainium2 Bass kernel for nn_BartCrossAttention (B=4, L=1024, D=1024, H=16, HD=64).

Sharding: 8 cores; core c handles query tokens [512c, 512c+512) of batch b=c//2.
K/V projections are split across the core pair by kv-token half (core 2j does
tokens 0-511, core 2j+1 tokens 512-1023 — the host slices kv accordingly) and
exchanged with pairwise AllGather collectives, halving the projection work.
The device program is identical on all cores: each computes its half into
local positions, gathers via DRAM bounce buffers, and reloads the full
rank-ordered result.

Other design points (see git history of this file for the evolution):
- All matmul operands bf16 (PE still 1 cycle/row; end-to-end rel err ~6e-3
  against the 2e-2 budget). hid/kv transposed on the HOST.
- ctx matmuls software-pipelined one t-iteration behind S/exp so the PE never
  waits on ACT inside an iteration.
- Softmax normalization fully off-PE: ones column in the ctx matmul produces
  the denominator row; DVE copies it out, gpsimd broadcasts, a 64-lane
  approx-reciprocal inverts it, and the PSUM->SBUF eviction multiplies.
- K projections for later head-pairs + Q projections interleave into earlier
  pairs' attention loops; during the last pair the first two out-projection
  chunks pre-accumulate fj=0..6.
- Bulk weight DMAs are token-gated (1-elem DVE copy into the dst tile = WAR
  dep) because the DMA engine round-robins all pending transfers.
"""
import sys

for _p in ("/opt/trn_rl_repo",):
    if _p not in sys.path:
        sys.path.insert(0, _p)

import numpy as np
import ml_dtypes

import concourse.bass as bass
import concourse.mybir as mybir
import concourse.tile as tile
from concourse import bacc
import concourse.bass_utils as bass_utils

F32 = mybir.dt.float32
BF16 = mybir.dt.bfloat16

P = 128
D = 1024        # model dim
H = 16          # heads
NCORES = 8
TQ = 512        # query tokens per core
LK = 1024       # kv tokens per batch
HK = 512        # kv tokens computed per core (half, exchanged with pair)
B, LQ = 4, 1024

PAIRS = [[0, 1], [2, 3], [4, 5], [6, 7]]

_CACHE = {}


def _build_core_program():
    nc = bacc.Bacc("TRN2", target_bir_lowering=False, debug=False,
                   num_devices=NCORES)

    hid_t = nc.dram_tensor("hid_t", [D, TQ], BF16, kind="ExternalInput")
    kv_t = nc.dram_tensor("kv_t", [D, HK], BF16, kind="ExternalInput")
    wq_t = nc.dram_tensor("wq_t", [D, D], BF16, kind="ExternalInput")
    wk_t = nc.dram_tensor("wk_t", [D, D], BF16, kind="ExternalInput")
    wv_t = nc.dram_tensor("wv_t", [D, D], BF16, kind="ExternalInput")
    wo_t = nc.dram_tensor("wo_t", [D, D], BF16, kind="ExternalInput")
    qb_d = nc.dram_tensor("qb", [D], F32, kind="ExternalInput")
    kb_d = nc.dram_tensor("kb", [D], F32, kind="ExternalInput")
    vb_d = nc.dram_tensor("vb", [D], F32, kind="ExternalInput")
    ob_d = nc.dram_tensor("ob", [D], F32, kind="ExternalInput")
    out_s = nc.dram_tensor("out_s", [TQ, D], F32, kind="ExternalOutput")

    Exp = mybir.ActivationFunctionType.Exp
    add = mybir.AluOpType.add
    mult = mybir.AluOpType.mult
    bypass = mybir.AluOpType.bypass

    with tile.TileContext(nc) as tc:
        with (
            tc.tile_pool(name="setup", bufs=1) as setup,
            tc.tile_pool(name="big", bufs=1) as big,
            tc.tile_pool(name="attn", bufs=4) as attnp,
            tc.tile_pool(name="norm", bufs=2) as normp,
            tc.tile_pool(name="outp", bufs=2) as outp,
            tc.tile_pool(name="dramp", bufs=1, space="DRAM") as dramp,
            tc.tile_pool(name="pssc", bufs=2, space="PSUM") as pssc,
            tc.tile_pool(name="psctx", bufs=4, space="PSUM") as psctx,
            tc.tile_pool(name="psmm", bufs=2, space="PSUM") as psmm,
        ):
            # ---- persistent big tiles ----
            kvT = big.tile([P, 8, HK], BF16, tag="kvT")      # my kv half ^T
            hidT = big.tile([P, 8, TQ], BF16, tag="hidT")    # hid^T [1024,512]
            wv = big.tile([P, 8, D], BF16, tag="wv")
            wk = big.tile([P, 8, D], BF16, tag="wk")
            wq = big.tile([P, 8, D], BF16, tag="wq")
            wo = big.tile([P, 8, D], BF16, tag="wo")
            KT = big.tile([P, 8, LK], BF16, tag="KT")        # K^T per pair
            qT = big.tile([P, 8, TQ], BF16, tag="qT")        # Q^T per pair
            v65 = big.tile([P, 8, H * 65], BF16, tag="v65")  # V+ones col
            ctxT = big.tile([P, 8, TQ], BF16, tag="ctxT")    # normalized ctx^T

            # DRAM bounce buffers for the pair AllGathers
            v_out = dramp.tile([P, 4, H * 65], BF16, tag="v_out")
            v_g = dramp.tile([2, P, 4, H * 65], BF16, tag="v_g")
            k_out = dramp.tile([P, 8, HK], BF16, tag="k_out")
            k_g = dramp.tile([2, P, 8, HK], BF16, tag="k_g")

            # ---- DMA dispatch order: first-needed first ----
            kv_re = kv_t.ap().rearrange("(dd p) t -> p dd t", p=P)
            wv_re = wv_t.ap().rearrange("(dd p) o -> p dd o", p=P)
            nc.sync.dma_start(kvT[:, :, 0:256], kv_re[:, :, 0:256])
            nc.sync.dma_start(wv[:, :, 0:512], wv_re[:, :, 0:512])
            nc.sync.dma_start(kvT[:, :, 256:512], kv_re[:, :, 256:512])

            # ---- setup: biases, ones, ACT table warm ----
            qb_sb = setup.tile([P, 8], F32, tag="qb")
            nc.sync.dma_start(qb_sb[:], qb_d.ap().rearrange("(o p) -> p o", p=P))
            kb_sb = setup.tile([P, 8], F32, tag="kb")
            nc.sync.dma_start(kb_sb[:], kb_d.ap().rearrange("(o p) -> p o", p=P))
            vbB = setup.tile([P, D], F32, tag="vbB")
            obB = setup.tile([P, D], F32, tag="obB")
            vb_row = setup.tile([1, D], F32, tag="vbrow")
            nc.sync.dma_start(vb_row[:], vb_d.ap()[None, :])
            nc.gpsimd.partition_broadcast(vbB[:], vb_row[:])
            ob_row = setup.tile([1, D], F32, tag="obrow")
            nc.sync.dma_start(ob_row[:], ob_d.ap()[None, :])
            nc.gpsimd.partition_broadcast(obB[:], ob_row[:])

            onesF = setup.tile([P, P], F32, tag="onesF")
            nc.gpsimd.memset(onesF[:], 1.0)
            warm = setup.tile([1, 8], BF16, tag="warm")
            nc.scalar.activation(warm[:], onesF[0:1, 0:8], Exp)

            # ones columns of v65 (col 64 of each head block)
            nc.vector.tensor_copy(
                v65[:].rearrange("p t (h x) -> p t h x", x=65)[:, :, :, 64:65],
                onesF[:].rearrange("p (t h x) -> p t h x", t=8, h=16))

            # ---- V projection: my 4 token tiles into local positions ----
            v65v = v65[:].rearrange("p t (h x) -> p t h x", x=65)

            def gated_dma(tok_ap, dst_ap, src_ap):
                # tok_ap MUST lie inside dst_ap so the DMA overwrites it
                nc.vector.tensor_copy(tok_ap, v65[0:1, 0, 0:1])
                nc.sync.dma_start(dst_ap, src_ap)

            for half in range(2):
                for ti in range(4):
                    pp = psmm.tile([P, 512], F32, tag="pp",
                                   name=f"ppv{half}_{ti}")
                    for di in range(8):
                        nc.tensor.matmul(
                            pp[:],
                            kvT[:, di, ti * P:(ti + 1) * P],
                            wv[:, di, half * 512:(half + 1) * 512],
                            start=(di == 0), stop=(di == 7),
                        )
                    nc.vector.tensor_tensor(
                        v65v[:, ti, half * 8:(half + 1) * 8, 0:64], pp[:],
                        vbB[:, half * 512:(half + 1) * 512], add)
                    if half == 0 and ti == 0:
                        gated_dma(wk[0:1, 0, 0:1], wk[:],
                                  wk_t.ap().rearrange("(dd p) o -> p dd o", p=P))
                    elif half == 0 and ti == 1:
                        gated_dma(wv[0:1, 0, 512:513], wv[:, :, 512:1024],
                                  wv_re[:, :, 512:1024])
                    elif half == 0 and ti == 3:
                        gated_dma(hidT[0:1, 0, 0:1], hidT[:],
                                  hid_t.ap().rearrange("(dd p) t -> p dd t", p=P))
                    elif half == 1 and ti == 1:
                        gated_dma(wq[0:1, 0, 0:1], wq[:],
                                  wq_t.ap().rearrange("(dd p) o -> p dd o", p=P))
                    elif half == 1 and ti == 3:
                        gated_dma(wo[0:1, 0, 0:1], wo[:],
                                  wo_t.ap().rearrange("(dd p) o -> p dd o", p=P))

            # V exchange: my half out, AllGather, full reload (rank order ==
            # token order since core 2j holds tokens 0-511)
            nc.sync.dma_start(v_out[:], v65[:, 0:4, :])
            nc.gpsimd.collective_compute(
                "AllGather", bypass, replica_groups=PAIRS,
                ins=[v_out[:].opt()], outs=[v_g[:].opt()])

            # ---- K projections: my token half, all head-pairs ----
            def emit_kproj(hp):
                pp = psmm.tile([P, 512], F32, tag="pp", name=f"ppk{hp}")
                for di in range(8):
                    nc.tensor.matmul(
                        pp[:],
                        wk[:, di, hp * P:(hp + 1) * P],
                        kvT[:, di, :],
                        start=(di == 0), stop=(di == 7),
                    )
                nc.vector.tensor_scalar(
                    KT[:, hp, 0:HK], pp[:],
                    kb_sb[:, hp:hp + 1], None, add)

            def emit_qproj(hp):
                pq = psmm.tile([P, 512], F32, tag="pp", name=f"ppq{hp}")
                for di in range(8):
                    nc.tensor.matmul(
                        pq[:],
                        wq[:, di, hp * P:(hp + 1) * P],
                        hidT[:, di, :],
                        start=(di == 0), stop=(di == 7),
                    )
                nc.vector.tensor_scalar(qT[:, hp, :], pq[:],
                                        qb_sb[:, hp:hp + 1], None, add)

            # all 8 head-pairs' K (my token half) while the V gather flies
            for hp in range(8):
                emit_kproj(hp)
            # V reload (plain per-rank slices; issued on gpsimd so they are
            # in-order AFTER the V collective on that engine's stream)
            nc.gpsimd.dma_start(v65[:, 0:4, :], v_g[0])
            nc.gpsimd.dma_start(v65[:, 4:8, :], v_g[1])
            nc.sync.dma_start(k_out[:], KT[:, :, 0:HK])
            nc.gpsimd.collective_compute(
                "AllGather", bypass, replica_groups=PAIRS,
                ins=[k_out[:].opt()], outs=[k_g[:].opt()])
            # Q0-3 cover the K gather; Q4-7 interleave into the main loop
            for hp in range(4):
                emit_qproj(hp)
            nc.gpsimd.dma_start(KT[:, :, 0:HK], k_g[0])
            nc.gpsimd.dma_start(KT[:, :, HK:LK], k_g[1])

            # normalization: all off-PE (DVE + tiny DMA + gpsimd + DVE)
            def emit_norm(hp, ctx_ps):
                srows = []
                for hh in range(2):
                    srow = normp.tile([65, 512], F32, tag="srow",
                                      name=f"srow{hp}_{hh}")
                    nc.vector.tensor_copy(srow[64:65, :], ctx_ps[hh][64:65, :])
                    srows.append(srow)
                rcs = []
                for hh in range(2):
                    r0 = normp.tile([1, 512], F32, tag="r0",
                                    name=f"r0_{hp}_{hh}")
                    nc.sync.dma_start(r0[:], srows[hh][64:65, :])
                    bc = normp.tile([64, 512], F32, tag="bc",
                                    name=f"bc{hp}_{hh}")
                    nc.gpsimd.partition_broadcast(bc[:], r0[:])
                    rc = normp.tile([64, 512], F32, tag="rc",
                                    name=f"rc{hp}_{hh}")
                    nc.vector.reciprocal_approx_fast(rc[:], bc[:])
                    rcs.append(rc)
                nc.vector.tensor_tensor(
                    ctxT[0:64, hp, :], ctx_ps[0][0:64, :], rcs[0][:], mult)
                stg = normp.tile([64, 512], BF16, tag="stg", name=f"stg{hp}")
                nc.vector.tensor_tensor(stg[:], ctx_ps[1][0:64, :],
                                        rcs[1][:], mult)
                nc.sync.dma_start(ctxT[64:128, hp, :], stg[:])

            # out-projection chunk helpers (epilogue + hp7 partials)
            def o_chunk_matmuls(po, half, mi, fjs, start0, stop7):
                for fj in fjs:
                    nc.tensor.matmul(
                        po[:],
                        ctxT[:, fj, mi * P:(mi + 1) * P],
                        wo[:, fj, half * 512:(half + 1) * 512],
                        start=(fj == 0 and start0), stop=(fj == 7 and stop7),
                    )

            def o_chunk_finish(po, half, mi):
                ot = outp.tile([P, 512], F32, tag="ot")
                nc.vector.tensor_tensor(
                    ot[:], po[:], obB[:, half * 512:(half + 1) * 512], add)
                nc.sync.dma_start(
                    out_s.ap().rearrange("(mm p) d -> p mm d", p=P)[
                        :, mi, half * 512:(half + 1) * 512],
                    ot[:])

            # ---- main attention loop (ctx pipelined 1 iter behind) ----
            ctx_tiles = {}
            pend = None
            opart = []

            def emit_ctx(hp, t, ats):
                for hh in range(2):
                    h = 2 * hp + hh
                    nc.tensor.matmul(
                        ctx_tiles[hp][hh][:],
                        v65[:, t, h * 65:(h + 1) * 65],
                        ats[hh][:],
                        start=(t == 0), stop=(t == 7),
                    )

            for hp in range(8):
                nxt = hp + 1
                ctx_tiles[hp] = [psctx.tile([65, 512], F32, tag="ctx",
                                            name=f"ctx{hp}_{i}")
                                 for i in range(2)]
                for t in range(8):
                    ats = []
                    for hh in range(2):
                        lo = 64 * hh
                        sc = pssc.tile([P, 512], F32, tag="sc",
                                       name=f"sc{hp}_{t}_{hh}")
                        nc.tensor.matmul(
                            sc[:],
                            KT[lo:lo + 64, hp, t * P:(t + 1) * P],
                            qT[lo:lo + 64, hp, :],
                            start=True, stop=True,
                        )
                        at = attnp.tile([P, 512], BF16, tag="at")
                        nc.scalar.activation(at[:], sc[:], Exp)
                        ats.append(at)
                    if pend is not None:
                        phh, pt, pats = pend
                        emit_ctx(phh, pt, pats)
                        if pt == 7:
                            emit_norm(phh, ctx_tiles[phh])
                    # interleaved projection / partial-O work
                    if hp < 4:
                        if t == 4:
                            emit_qproj(hp + 4)
                    elif hp == 7:
                        if t == 2 or t == 4:
                            mi = 0 if t == 2 else 1
                            po = psmm.tile([P, 512], F32, tag="pp",
                                           name=f"ppo0_{mi}")
                            o_chunk_matmuls(po, 0, mi, range(7), True, False)
                            opart.append((po, 0, mi))
                    pend = (hp, t, ats)

            emit_ctx(7, 7, pend[2])
            emit_norm(7, ctx_tiles[7])

            # ---- epilogue: finish out projection ----
            for po, half, mi in opart:
                o_chunk_matmuls(po, half, mi, [7], False, True)
                o_chunk_finish(po, half, mi)
            for half in range(2):
                for mi in range(4):
                    if half == 0 and mi < 2:
                        continue
                    po = psmm.tile([P, 512], F32, tag="pp",
                                   name=f"ppo{half}_{mi}")
                    o_chunk_matmuls(po, half, mi, range(8), True, True)
                    o_chunk_finish(po, half, mi)

    nc.compile()
    return nc


def _prep_inputs(hidden_states, key_value_states, q_weight, q_bias,
                 kv_weight, kv_bias, out_weight, out_bias):
    f32 = np.float32
    bf16 = ml_dtypes.bfloat16
    hid = np.asarray(hidden_states, f32).reshape(B * LQ, D)
    kv = np.asarray(key_value_states, f32).reshape(B * LK, D)
    scale = f32(1.0 / 8.0)

    # de-interleave kv rows: row e <-> (h=e//128, j=(e%128)//64, d=e%64)
    e = np.arange(2 * D)
    kmask = (e % 128) < 64
    kidx, vidx = e[kmask], e[~kmask]
    kvw = np.asarray(kv_weight, f32)
    kvb = np.asarray(kv_bias, f32)

    shared = {
        "wq_t": np.ascontiguousarray((np.asarray(q_weight, f32) * scale).T.astype(bf16)),
        "wk_t": np.ascontiguousarray(kvw[kidx].T.astype(bf16)),
        "wv_t": np.ascontiguousarray(kvw[vidx].T.astype(bf16)),
        "wo_t": np.ascontiguousarray(np.asarray(out_weight, f32).T.astype(bf16)),
        "qb": np.ascontiguousarray(np.asarray(q_bias, f32) * scale),
        "kb": np.ascontiguousarray(kvb[kidx]),
        "vb": np.ascontiguousarray(kvb[vidx]),
        "ob": np.ascontiguousarray(np.asarray(out_bias, f32)),
    }
    kvT_halves = {}
    for b in range(B):
        kb_full = kv[b * LK:(b + 1) * LK].T.astype(bf16)  # [D, LK]
        kvT_halves[(b, 0)] = np.ascontiguousarray(kb_full[:, 0:HK])
        kvT_halves[(b, 1)] = np.ascontiguousarray(kb_full[:, HK:LK])
    in_maps = []
    for c in range(NCORES):
        b = c // 2
        m = dict(shared)
        m["hid_t"] = np.ascontiguousarray(
            hid[c * TQ:(c + 1) * TQ].T.astype(bf16))
        m["kv_t"] = kvT_halves[(b, c % 2)]
        in_maps.append(m)
    return in_maps


def kernel(hidden_states, key_value_states, q_weight, q_bias,
           kv_weight, kv_bias, out_weight, out_bias, _trace=False):
    if "nc" not in _CACHE:
        _CACHE["nc"] = _build_core_program()
    nc = _CACHE["nc"]
    in_maps = _prep_inputs(hidden_states, key_value_states, q_weight, q_bias,
                           kv_weight, kv_bias, out_weight, out_bias)
    res = bass_utils.run_bass_kernel_spmd(
        nc, in_maps, core_ids=list(range(NCORES)), trace=_trace)
    _CACHE["last_result"] = res
    out = np.concatenate([r["out_s"] for r in res.results], axis=0)
    return out.reshape(B, LQ, D)


# revision 24
# speedup vs baseline: 1.4391x; 1.4391x over previous
"""Trainium2 Bass kernel for nn_BartCrossAttention (B=4, L=1024, D=1024, H=16, HD=64).

Sharding: 8 cores; core c handles query tokens [512c, 512c+512) of batch b=c//2.
Each core recomputes K/V projections for its whole batch (no collectives); the
host slices inputs per core and concatenates outputs.

Design notes:
- All matmul operands bf16 (PE still 1 cycle/row, halves DMA+SBUF traffic;
  measured end-to-end rel err ~6e-3 vs the 2e-2 budget).
- hid/kv transposed on the HOST - no on-device PE transposes at all.
- ctx matmuls are software-pipelined one t-iteration behind S/exp so the PE
  never waits on the ACT engine inside an iteration.
- Softmax normalization is off-PE: ones column in the ctx matmul gives the
  denominator row, reciprocal_approx_fast on DVE, gpsimd partition_broadcast,
  and the PSUM->SBUF ctx eviction does the multiply (normalize-on-evict).
- K/Q projections for pair hp+1 interleave into pair hp's t-loop; during the
  last pair (no projections left) the first two out-projection chunks
  pre-accumulate fj=0..6 so the PE stays fed while ACT drains.
- DMA dispatches are ordered first-needed-first (each dma_start costs ~1us of
  Sync dispatch); ACT exp table is pre-warmed in the prologue.
"""
import sys

for _p in ("/opt/trn_rl_repo",):
    if _p not in sys.path:
        sys.path.insert(0, _p)

import numpy as np
import ml_dtypes

import concourse.bass as bass
import concourse.mybir as mybir
import concourse.tile as tile
from concourse import bacc
import concourse.bass_utils as bass_utils

F32 = mybir.dt.float32
BF16 = mybir.dt.bfloat16

P = 128
D = 1024        # model dim
H = 16          # heads
NCORES = 8
TQ = 512        # query tokens per core
LK = 1024       # kv tokens per batch
B, LQ = 4, 1024

_CACHE = {}


def _build_core_program():
    nc = bacc.Bacc("TRN2", target_bir_lowering=False, debug=False,
                   num_devices=NCORES)

    hid_t = nc.dram_tensor("hid_t", [D, TQ], BF16, kind="ExternalInput")
    kv_t = nc.dram_tensor("kv_t", [D, LK], BF16, kind="ExternalInput")
    wq_t = nc.dram_tensor("wq_t", [D, D], BF16, kind="ExternalInput")
    wk_t = nc.dram_tensor("wk_t", [D, D], BF16, kind="ExternalInput")
    wv_t = nc.dram_tensor("wv_t", [D, D], BF16, kind="ExternalInput")
    wo_t = nc.dram_tensor("wo_t", [D, D], BF16, kind="ExternalInput")
    qb_d = nc.dram_tensor("qb", [D], F32, kind="ExternalInput")
    kb_d = nc.dram_tensor("kb", [D], F32, kind="ExternalInput")
    vb_d = nc.dram_tensor("vb", [D], F32, kind="ExternalInput")
    ob_d = nc.dram_tensor("ob", [D], F32, kind="ExternalInput")
    out_s = nc.dram_tensor("out_s", [TQ, D], F32, kind="ExternalOutput")

    Exp = mybir.ActivationFunctionType.Exp
    add = mybir.AluOpType.add
    mult = mybir.AluOpType.mult

    with tile.TileContext(nc) as tc:
        with (
            tc.tile_pool(name="setup", bufs=1) as setup,
            tc.tile_pool(name="big", bufs=1) as big,
            tc.tile_pool(name="attn", bufs=4) as attnp,
            tc.tile_pool(name="norm", bufs=2) as normp,
            tc.tile_pool(name="outp", bufs=2) as outp,
            tc.tile_pool(name="pssc", bufs=2, space="PSUM") as pssc,
            tc.tile_pool(name="psctx", bufs=4, space="PSUM") as psctx,
            tc.tile_pool(name="psmm", bufs=2, space="PSUM") as psmm,
        ):
            # ---- persistent big tiles ----
            kvT = big.tile([P, 8, LK], BF16, tag="kvT")      # kv^T [1024,1024]
            hidT = big.tile([P, 8, TQ], BF16, tag="hidT")    # hid^T [1024,512]
            wv = big.tile([P, 8, D], BF16, tag="wv")
            wk = big.tile([P, 8, D], BF16, tag="wk")
            wq = big.tile([P, 8, D], BF16, tag="wq")
            wo = big.tile([P, 8, D], BF16, tag="wo")
            KT = big.tile([P, 8, LK], BF16, tag="KT")        # K^T per pair
            qT = big.tile([P, 8, TQ], BF16, tag="qT")        # Q^T per pair
            v65 = big.tile([P, 8, H * 65], BF16, tag="v65")  # V+ones col
            ctxT = big.tile([P, 8, TQ], BF16, tag="ctxT")    # normalized ctx^T

            # ---- DMA dispatch order: first-needed first. The DMA engine
            # round-robins all pending transfers, so later bulk loads are
            # token-gated (1-elem DVE copy into the dst tile = WAR dep) to
            # keep them from stealing bandwidth from the critical prologue.
            kv_re = kv_t.ap().rearrange("(dd p) t -> p dd t", p=P)
            wv_re = wv_t.ap().rearrange("(dd p) o -> p dd o", p=P)
            nc.sync.dma_start(kvT[:, :, 0:256], kv_re[:, :, 0:256])
            nc.sync.dma_start(wv[:, 0:4, 0:512], wv_re[:, 0:4, 0:512])
            nc.sync.dma_start(wv[:, 4:8, 0:512], wv_re[:, 4:8, 0:512])
            nc.sync.dma_start(kvT[:, :, 256:512], kv_re[:, :, 256:512])
            nc.sync.dma_start(kvT[:, :, 512:1024], kv_re[:, :, 512:1024])
            nc.sync.dma_start(wv[:, :, 512:1024], wv_re[:, :, 512:1024])

            # ---- setup: biases, ones, ACT table warm ----
            qb_sb = setup.tile([P, 8], F32, tag="qb")
            nc.sync.dma_start(qb_sb[:], qb_d.ap().rearrange("(o p) -> p o", p=P))
            kb_sb = setup.tile([P, 8], F32, tag="kb")
            nc.sync.dma_start(kb_sb[:], kb_d.ap().rearrange("(o p) -> p o", p=P))
            vbB = setup.tile([P, D], F32, tag="vbB")
            obB = setup.tile([P, D], F32, tag="obB")
            vb_row = setup.tile([1, D], F32, tag="vbrow")
            nc.sync.dma_start(vb_row[:], vb_d.ap()[None, :])
            nc.gpsimd.partition_broadcast(vbB[:], vb_row[:])
            ob_row = setup.tile([1, D], F32, tag="obrow")
            nc.sync.dma_start(ob_row[:], ob_d.ap()[None, :])
            nc.gpsimd.partition_broadcast(obB[:], ob_row[:])

            onesF = setup.tile([P, P], F32, tag="onesF")
            nc.gpsimd.memset(onesF[:], 1.0)
            warm = setup.tile([1, 8], BF16, tag="warm")
            nc.scalar.activation(warm[:], onesF[0:1, 0:8], Exp)

            # ones columns of v65 (col 64 of each head block)
            nc.vector.tensor_copy(
                v65[:].rearrange("p t (h x) -> p t h x", x=65)[:, :, :, 64:65],
                onesF[:].rearrange("p (t h x) -> p t h x", t=8, h=16))

            # ---- V projection (with token-gated weight loads) ----
            v65v = v65[:].rearrange("p t (h x) -> p t h x", x=65)

            def gated_dma(dst_tile, dst_ap, src_ap):
                nc.vector.tensor_copy(dst_tile[0:1, 0, 0:1],
                                      v65[0:1, 0, 0:1])
                nc.sync.dma_start(dst_ap, src_ap)

            for half in range(2):
                for ti in range(8):
                    pp = psmm.tile([P, 512], F32, tag="pp",
                                   name=f"ppv{half}_{ti}")
                    for di in range(8):
                        nc.tensor.matmul(
                            pp[:],
                            kvT[:, di, ti * P:(ti + 1) * P],
                            wv[:, di, half * 512:(half + 1) * 512],
                            start=(di == 0), stop=(di == 7),
                        )
                    nc.vector.tensor_tensor(
                        v65v[:, ti, half * 8:(half + 1) * 8, 0:64], pp[:],
                        vbB[:, half * 512:(half + 1) * 512], add)
                    if half == 0 and ti == 7:
                        gated_dma(wk, wk[:],
                                  wk_t.ap().rearrange("(dd p) o -> p dd o", p=P))
                    elif half == 1 and ti == 1:
                        gated_dma(hidT, hidT[:],
                                  hid_t.ap().rearrange("(dd p) t -> p dd t", p=P))
                    elif half == 1 and ti == 4:
                        gated_dma(wq, wq[:],
                                  wq_t.ap().rearrange("(dd p) o -> p dd o", p=P))
                    elif half == 1 and ti == 7:
                        gated_dma(wo, wo[:],
                                  wo_t.ap().rearrange("(dd p) o -> p dd o", p=P))

            # ---- K/Q projections (pair 0 now, rest interleaved) ----
            def emit_kproj(hp, nk):
                pp = psmm.tile([P, 512], F32, tag="pp", name=f"ppk{hp}_{nk}")
                for di in range(8):
                    nc.tensor.matmul(
                        pp[:],
                        wk[:, di, hp * P:(hp + 1) * P],
                        kvT[:, di, nk * 512:(nk + 1) * 512],
                        start=(di == 0), stop=(di == 7),
                    )
                nc.vector.tensor_scalar(
                    KT[:, hp, nk * 512:(nk + 1) * 512], pp[:],
                    kb_sb[:, hp:hp + 1], None, add)

            def emit_qproj(hp):
                pq = psmm.tile([P, 512], F32, tag="pp", name=f"ppq{hp}")
                for di in range(8):
                    nc.tensor.matmul(
                        pq[:],
                        wq[:, di, hp * P:(hp + 1) * P],
                        hidT[:, di, :],
                        start=(di == 0), stop=(di == 7),
                    )
                nc.vector.tensor_scalar(qT[:, hp, :], pq[:],
                                        qb_sb[:, hp:hp + 1], None, add)

            emit_kproj(0, 0)
            emit_kproj(0, 1)
            emit_qproj(0)

            # normalization: all off-PE. Copy the sums row out of PSUM, DMA it
            # to partition 0, gpsimd-broadcast the raw sums, then a 64-lane
            # approx reciprocal (18-bit accurate, plenty for well-conditioned
            # positive denominators) and normalize-on-evict.
            def emit_norm(hp, ctx_ps):
                # both heads' chains interleaved to overlap the serial
                # copy -> DMA -> broadcast -> reciprocal -> multiply latency
                srows, r0s, bcs, rcs = [], [], [], []
                for hh in range(2):
                    srow = normp.tile([65, 512], F32, tag="srow",
                                      name=f"srow{hp}_{hh}")
                    nc.vector.tensor_copy(srow[64:65, :], ctx_ps[hh][64:65, :])
                    srows.append(srow)
                for hh in range(2):
                    r0 = normp.tile([1, 512], F32, tag="r0",
                                    name=f"r0_{hp}_{hh}")
                    nc.sync.dma_start(r0[:], srows[hh][64:65, :])
                    r0s.append(r0)
                for hh in range(2):
                    bc = normp.tile([64, 512], F32, tag="bc",
                                    name=f"bc{hp}_{hh}")
                    nc.gpsimd.partition_broadcast(bc[:], r0s[hh][:])
                    bcs.append(bc)
                for hh in range(2):
                    rc = normp.tile([64, 512], F32, tag="rc",
                                    name=f"rc{hp}_{hh}")
                    nc.vector.reciprocal_approx_fast(rc[:], bcs[hh][:])
                    rcs.append(rc)
                nc.vector.tensor_tensor(
                    ctxT[0:64, hp, :], ctx_ps[0][0:64, :], rcs[0][:], mult)
                stg = normp.tile([64, 512], BF16, tag="stg", name=f"stg{hp}")
                nc.vector.tensor_tensor(stg[:], ctx_ps[1][0:64, :],
                                        rcs[1][:], mult)
                nc.sync.dma_start(ctxT[64:128, hp, :], stg[:])

            # out-projection chunk helpers (epilogue + hp7 partials)
            def o_chunk_matmuls(po, half, mi, fjs, start0, stop7):
                for fj in fjs:
                    nc.tensor.matmul(
                        po[:],
                        ctxT[:, fj, mi * P:(mi + 1) * P],
                        wo[:, fj, half * 512:(half + 1) * 512],
                        start=(fj == 0 and start0), stop=(fj == 7 and stop7),
                    )

            def o_chunk_finish(po, half, mi):
                ot = outp.tile([P, 512], F32, tag="ot")
                nc.vector.tensor_tensor(
                    ot[:], po[:], obB[:, half * 512:(half + 1) * 512], add)
                nc.sync.dma_start(
                    out_s.ap().rearrange("(mm p) d -> p mm d", p=P)[
                        :, mi, half * 512:(half + 1) * 512],
                    ot[:])

            # ---- main attention loop (ctx pipelined 1 iter behind) ----
            ctx_tiles = {}
            pend = None  # (hp, t, [at_e, at_o])
            opart = []   # hp7 partial out-proj chunks: (po, half, mi)

            def emit_ctx(hp, t, ats):
                for hh in range(2):
                    h = 2 * hp + hh
                    nc.tensor.matmul(
                        ctx_tiles[hp][hh][:],
                        v65[:, t, h * 65:(h + 1) * 65],
                        ats[hh][:],
                        start=(t == 0), stop=(t == 7),
                    )

            for hp in range(8):
                nxt = hp + 1
                ctx_tiles[hp] = [psctx.tile([65, 512], F32, tag="ctx",
                                            name=f"ctx{hp}_{i}")
                                 for i in range(2)]
                for t in range(8):
                    ats = []
                    for hh in range(2):
                        lo = 64 * hh
                        sc = pssc.tile([P, 512], F32, tag="sc",
                                       name=f"sc{hp}_{t}_{hh}")
                        nc.tensor.matmul(
                            sc[:],
                            KT[lo:lo + 64, hp, t * P:(t + 1) * P],
                            qT[lo:lo + 64, hp, :],
                            start=True, stop=True,
                        )
                        at = attnp.tile([P, 512], BF16, tag="at")
                        nc.scalar.activation(at[:], sc[:], Exp)
                        ats.append(at)
                    if pend is not None:
                        phh, pt, pats = pend
                        emit_ctx(phh, pt, pats)
                        if pt == 7:
                            emit_norm(phh, ctx_tiles[phh])
                    if nxt < 8:
                        if t == 2:
                            emit_kproj(nxt, 0)
                        elif t == 4:
                            emit_kproj(nxt, 1)
                        elif t == 6:
                            emit_qproj(nxt)
                    else:
                        # keep PE fed while ACT drains: pre-accumulate the
                        # first two out-proj chunks over fj=0..6
                        if t == 2 or t == 4:
                            mi = 0 if t == 2 else 1
                            po = psmm.tile([P, 512], F32, tag="pp",
                                           name=f"ppo0_{mi}")
                            o_chunk_matmuls(po, 0, mi, range(7), True, False)
                            opart.append((po, 0, mi))
                    pend = (hp, t, ats)

            emit_ctx(7, 7, pend[2])
            emit_norm(7, ctx_tiles[7])

            # ---- epilogue: finish out projection ----
            for po, half, mi in opart:
                o_chunk_matmuls(po, half, mi, [7], False, True)
                o_chunk_finish(po, half, mi)
            for half in range(2):
                for mi in range(4):
                    if half == 0 and mi < 2:
                        continue
                    po = psmm.tile([P, 512], F32, tag="pp",
                                   name=f"ppo{half}_{mi}")
                    o_chunk_matmuls(po, half, mi, range(8), True, True)
                    o_chunk_finish(po, half, mi)

    nc.compile()
    return nc


def _prep_inputs(hidden_states, key_value_states, q_weight, q_bias,
                 kv_weight, kv_bias, out_weight, out_bias):
    f32 = np.float32
    bf16 = ml_dtypes.bfloat16
    hid = np.asarray(hidden_states, f32).reshape(B * LQ, D)
    kv = np.asarray(key_value_states, f32).reshape(B * LK, D)
    scale = f32(1.0 / 8.0)

    # de-interleave kv rows: row e <-> (h=e//128, j=(e%128)//64, d=e%64)
    e = np.arange(2 * D)
    kmask = (e % 128) < 64
    kidx, vidx = e[kmask], e[~kmask]
    kvw = np.asarray(kv_weight, f32)
    kvb = np.asarray(kv_bias, f32)

    shared = {
        "wq_t": np.ascontiguousarray((np.asarray(q_weight, f32) * scale).T.astype(bf16)),
        "wk_t": np.ascontiguousarray(kvw[kidx].T.astype(bf16)),
        "wv_t": np.ascontiguousarray(kvw[vidx].T.astype(bf16)),
        "wo_t": np.ascontiguousarray(np.asarray(out_weight, f32).T.astype(bf16)),
        "qb": np.ascontiguousarray(np.asarray(q_bias, f32) * scale),
        "kb": np.ascontiguousarray(kvb[kidx]),
        "vb": np.ascontiguousarray(kvb[vidx]),
        "ob": np.ascontiguousarray(np.asarray(out_bias, f32)),
    }
    kvT_by_batch = [
        np.ascontiguousarray(kv[b * LK:(b + 1) * LK].T.astype(bf16))
        for b in range(B)
    ]
    in_maps = []
    for c in range(NCORES):
        b = c // 2
        m = dict(shared)
        m["hid_t"] = np.ascontiguousarray(
            hid[c * TQ:(c + 1) * TQ].T.astype(bf16))
        m["kv_t"] = kvT_by_batch[b]
        in_maps.append(m)
    return in_maps


def kernel(hidden_states, key_value_states, q_weight, q_bias,
           kv_weight, kv_bias, out_weight, out_bias, _trace=False):
    if "nc" not in _CACHE:
        _CACHE["nc"] = _build_core_program()
    nc = _CACHE["nc"]
    in_maps = _prep_inputs(hidden_states, key_value_states, q_weight, q_bias,
                           kv_weight, kv_bias, out_weight, out_bias)
    res = bass_utils.run_bass_kernel_spmd(
        nc, in_maps, core_ids=list(range(NCORES)), trace=_trace)
    _CACHE["last_result"] = res
    out = np.concatenate([r["out_s"] for r in res.results], axis=0)
    return out.reshape(B, LQ, D)


# revision 26
# speedup vs baseline: 1.4660x; 1.0187x over previous
"""Trainium2 Bass kernel for nn_BartCrossAttention (B=4, L=1024, D=1024, H=16, HD=64).

Sharding: 8 cores; core c handles query tokens [512c, 512c+512) of batch b=c//2.
Each core recomputes K/V projections for its whole batch (no collectives); the
host slices inputs per core and concatenates outputs.

Design notes:
- All matmul operands bf16 (PE still 1 cycle/row, halves DMA+SBUF traffic;
  measured end-to-end rel err ~6e-3 vs the 2e-2 budget).
- hid/kv transposed on the HOST - no on-device PE transposes at all.
- ctx matmuls are software-pipelined one t-iteration behind S/exp so the PE
  never waits on the ACT engine inside an iteration.
- Softmax normalization is off-PE: ones column in the ctx matmul gives the
  denominator row, reciprocal_approx_fast on DVE, gpsimd partition_broadcast,
  and the PSUM->SBUF ctx eviction does the multiply (normalize-on-evict).
- K/Q projections for pair hp+1 interleave into pair hp's t-loop; during the
  last pair (no projections left) the first two out-projection chunks
  pre-accumulate fj=0..6 so the PE stays fed while ACT drains.
- DMA dispatches are ordered first-needed-first (each dma_start costs ~1us of
  Sync dispatch); ACT exp table is pre-warmed in the prologue.
"""
import sys

for _p in ("/opt/trn_rl_repo",):
    if _p not in sys.path:
        sys.path.insert(0, _p)

import numpy as np
import ml_dtypes

import concourse.bass as bass
import concourse.mybir as mybir
import concourse.tile as tile
from concourse import bacc
import concourse.bass_utils as bass_utils

F32 = mybir.dt.float32
BF16 = mybir.dt.bfloat16

P = 128
D = 1024        # model dim
H = 16          # heads
NCORES = 8
TQ = 512        # query tokens per core
LK = 1024       # kv tokens per batch
B, LQ = 4, 1024

_CACHE = {}


def _build_core_program():
    nc = bacc.Bacc("TRN2", target_bir_lowering=False, debug=False,
                   num_devices=NCORES)

    hid_t = nc.dram_tensor("hid_t", [D, TQ], BF16, kind="ExternalInput")
    kv_t = nc.dram_tensor("kv_t", [D, LK], BF16, kind="ExternalInput")
    wq_t = nc.dram_tensor("wq_t", [D, D], BF16, kind="ExternalInput")
    wk_t = nc.dram_tensor("wk_t", [D, D], BF16, kind="ExternalInput")
    wv_t = nc.dram_tensor("wv_t", [D, D], BF16, kind="ExternalInput")
    wo_t = nc.dram_tensor("wo_t", [D, D], BF16, kind="ExternalInput")
    qb_d = nc.dram_tensor("qb", [D], F32, kind="ExternalInput")
    kb_d = nc.dram_tensor("kb", [D], F32, kind="ExternalInput")
    vb_d = nc.dram_tensor("vb", [D], F32, kind="ExternalInput")
    ob_d = nc.dram_tensor("ob", [D], F32, kind="ExternalInput")
    out_s = nc.dram_tensor("out_s", [TQ, D], F32, kind="ExternalOutput")

    Exp = mybir.ActivationFunctionType.Exp
    add = mybir.AluOpType.add
    mult = mybir.AluOpType.mult

    with tile.TileContext(nc) as tc:
        with (
            tc.tile_pool(name="setup", bufs=1) as setup,
            tc.tile_pool(name="big", bufs=1) as big,
            tc.tile_pool(name="attn", bufs=4) as attnp,
            tc.tile_pool(name="norm", bufs=2) as normp,
            tc.tile_pool(name="outp", bufs=2) as outp,
            tc.tile_pool(name="pssc", bufs=2, space="PSUM") as pssc,
            tc.tile_pool(name="psctx", bufs=4, space="PSUM") as psctx,
            tc.tile_pool(name="psmm", bufs=2, space="PSUM") as psmm,
        ):
            # ---- persistent big tiles ----
            kvT = big.tile([P, 8, LK], BF16, tag="kvT")      # kv^T [1024,1024]
            hidT = big.tile([P, 8, TQ], BF16, tag="hidT")    # hid^T [1024,512]
            wv = big.tile([P, 8, D], BF16, tag="wv")
            wk = big.tile([P, 8, D], BF16, tag="wk")
            wq = big.tile([P, 8, D], BF16, tag="wq")
            wo = big.tile([P, 8, D], BF16, tag="wo")
            KT = big.tile([P, 8, LK], BF16, tag="KT")        # K^T per pair
            qT = big.tile([P, 8, TQ], BF16, tag="qT")        # Q^T per pair
            v65 = big.tile([P, 8, H * 65], BF16, tag="v65")  # V+ones col
            ctxT = big.tile([P, 8, TQ], BF16, tag="ctxT")    # normalized ctx^T

            # ---- DMA dispatch order: first-needed first. The DMA engine
            # round-robins all pending transfers, so later bulk loads are
            # token-gated (1-elem DVE copy into the dst tile = WAR dep) to
            # keep them from stealing bandwidth from the critical prologue.
            # kvT as ONE whole-tile DMA: full 2KB dram lines (peak DMA
            # efficiency) and the whole tile resident before V-proj needs
            # its later token tiles
            kv_re = kv_t.ap().rearrange("(dd p) t -> p dd t", p=P)
            wv_re = wv_t.ap().rearrange("(dd p) o -> p dd o", p=P)
            nc.sync.dma_start(kvT[:], kv_re)
            nc.sync.dma_start(wv[:, :, 0:512], wv_re[:, :, 0:512])
            nc.sync.dma_start(wv[:, :, 512:1024], wv_re[:, :, 512:1024])

            # ---- setup: biases, ones, ACT table warm ----
            qb_sb = setup.tile([P, 8], F32, tag="qb")
            nc.sync.dma_start(qb_sb[:], qb_d.ap().rearrange("(o p) -> p o", p=P))
            kb_sb = setup.tile([P, 8], F32, tag="kb")
            nc.sync.dma_start(kb_sb[:], kb_d.ap().rearrange("(o p) -> p o", p=P))
            vbB = setup.tile([P, D], F32, tag="vbB")
            obB = setup.tile([P, D], F32, tag="obB")
            vb_row = setup.tile([1, D], F32, tag="vbrow")
            nc.sync.dma_start(vb_row[:], vb_d.ap()[None, :])
            nc.gpsimd.partition_broadcast(vbB[:], vb_row[:])
            ob_row = setup.tile([1, D], F32, tag="obrow")
            nc.sync.dma_start(ob_row[:], ob_d.ap()[None, :])
            nc.gpsimd.partition_broadcast(obB[:], ob_row[:])

            onesF = setup.tile([P, P], F32, tag="onesF")
            nc.gpsimd.memset(onesF[:], 1.0)
            warm = setup.tile([1, 8], BF16, tag="warm")
            nc.scalar.activation(warm[:], onesF[0:1, 0:8], Exp)

            # ones columns of v65 (col 64 of each head block)
            nc.vector.tensor_copy(
                v65[:].rearrange("p t (h x) -> p t h x", x=65)[:, :, :, 64:65],
                onesF[:].rearrange("p (t h x) -> p t h x", t=8, h=16))

            # ---- V projection (with token-gated weight loads) ----
            v65v = v65[:].rearrange("p t (h x) -> p t h x", x=65)

            def gated_dma(dst_tile, dst_ap, src_ap):
                nc.vector.tensor_copy(dst_tile[0:1, 0, 0:1],
                                      v65[0:1, 0, 0:1])
                nc.sync.dma_start(dst_ap, src_ap)

            for half in range(2):
                for ti in range(8):
                    pp = psmm.tile([P, 512], F32, tag="pp",
                                   name=f"ppv{half}_{ti}")
                    for di in range(8):
                        nc.tensor.matmul(
                            pp[:],
                            kvT[:, di, ti * P:(ti + 1) * P],
                            wv[:, di, half * 512:(half + 1) * 512],
                            start=(di == 0), stop=(di == 7),
                        )
                    nc.vector.tensor_tensor(
                        v65v[:, ti, half * 8:(half + 1) * 8, 0:64], pp[:],
                        vbB[:, half * 512:(half + 1) * 512], add)
                    if half == 0 and ti == 7:
                        gated_dma(wk, wk[:],
                                  wk_t.ap().rearrange("(dd p) o -> p dd o", p=P))
                    elif half == 1 and ti == 1:
                        gated_dma(hidT, hidT[:],
                                  hid_t.ap().rearrange("(dd p) t -> p dd t", p=P))
                    elif half == 1 and ti == 4:
                        gated_dma(wq, wq[:],
                                  wq_t.ap().rearrange("(dd p) o -> p dd o", p=P))
                    elif half == 1 and ti == 7:
                        gated_dma(wo, wo[:],
                                  wo_t.ap().rearrange("(dd p) o -> p dd o", p=P))

            # ---- K/Q projections (pair 0 now, rest interleaved) ----
            def emit_kproj(hp, nk):
                pp = psmm.tile([P, 512], F32, tag="pp", name=f"ppk{hp}_{nk}")
                for di in range(8):
                    nc.tensor.matmul(
                        pp[:],
                        wk[:, di, hp * P:(hp + 1) * P],
                        kvT[:, di, nk * 512:(nk + 1) * 512],
                        start=(di == 0), stop=(di == 7),
                    )
                nc.vector.tensor_scalar(
                    KT[:, hp, nk * 512:(nk + 1) * 512], pp[:],
                    kb_sb[:, hp:hp + 1], None, add)

            def emit_qproj(hp):
                pq = psmm.tile([P, 512], F32, tag="pp", name=f"ppq{hp}")
                for di in range(8):
                    nc.tensor.matmul(
                        pq[:],
                        wq[:, di, hp * P:(hp + 1) * P],
                        hidT[:, di, :],
                        start=(di == 0), stop=(di == 7),
                    )
                nc.vector.tensor_scalar(qT[:, hp, :], pq[:],
                                        qb_sb[:, hp:hp + 1], None, add)

            emit_kproj(0, 0)
            emit_kproj(0, 1)
            emit_qproj(0)

            # normalization: all off-PE. Copy the sums row out of PSUM, DMA it
            # to partition 0, gpsimd-broadcast the raw sums, then a 64-lane
            # approx reciprocal (18-bit accurate, plenty for well-conditioned
            # positive denominators) and normalize-on-evict.
            def emit_norm(hp, ctx_ps):
                # both heads' chains interleaved to overlap the serial
                # copy -> DMA -> broadcast -> reciprocal -> multiply latency
                srows, r0s, bcs, rcs = [], [], [], []
                for hh in range(2):
                    srow = normp.tile([65, 512], F32, tag="srow",
                                      name=f"srow{hp}_{hh}")
                    nc.vector.tensor_copy(srow[64:65, :], ctx_ps[hh][64:65, :])
                    srows.append(srow)
                for hh in range(2):
                    r0 = normp.tile([1, 512], F32, tag="r0",
                                    name=f"r0_{hp}_{hh}")
                    nc.sync.dma_start(r0[:], srows[hh][64:65, :])
                    r0s.append(r0)
                for hh in range(2):
                    bc = normp.tile([64, 512], F32, tag="bc",
                                    name=f"bc{hp}_{hh}")
                    nc.gpsimd.partition_broadcast(bc[:], r0s[hh][:])
                    bcs.append(bc)
                for hh in range(2):
                    rc = normp.tile([64, 512], F32, tag="rc",
                                    name=f"rc{hp}_{hh}")
                    nc.vector.reciprocal_approx_fast(rc[:], bcs[hh][:])
                    rcs.append(rc)
                nc.vector.tensor_tensor(
                    ctxT[0:64, hp, :], ctx_ps[0][0:64, :], rcs[0][:], mult)
                stg = normp.tile([64, 512], BF16, tag="stg", name=f"stg{hp}")
                nc.vector.tensor_tensor(stg[:], ctx_ps[1][0:64, :],
                                        rcs[1][:], mult)
                nc.sync.dma_start(ctxT[64:128, hp, :], stg[:])

            # out-projection chunk helpers (epilogue + hp7 partials)
            def o_chunk_matmuls(po, half, mi, fjs, start0, stop7):
                for fj in fjs:
                    nc.tensor.matmul(
                        po[:],
                        ctxT[:, fj, mi * P:(mi + 1) * P],
                        wo[:, fj, half * 512:(half + 1) * 512],
                        start=(fj == 0 and start0), stop=(fj == 7 and stop7),
                    )

            def o_chunk_finish(po, half, mi):
                ot = outp.tile([P, 512], F32, tag="ot")
                nc.vector.tensor_tensor(
                    ot[:], po[:], obB[:, half * 512:(half + 1) * 512], add)
                nc.sync.dma_start(
                    out_s.ap().rearrange("(mm p) d -> p mm d", p=P)[
                        :, mi, half * 512:(half + 1) * 512],
                    ot[:])

            # ---- main attention loop (ctx pipelined 1 iter behind) ----
            ctx_tiles = {}
            pend = None  # (hp, t, [at_e, at_o])
            opart = []   # hp7 partial out-proj chunks: (po, half, mi)

            def emit_ctx(hp, t, ats):
                for hh in range(2):
                    h = 2 * hp + hh
                    nc.tensor.matmul(
                        ctx_tiles[hp][hh][:],
                        v65[:, t, h * 65:(h + 1) * 65],
                        ats[hh][:],
                        start=(t == 0), stop=(t == 7),
                    )

            for hp in range(8):
                nxt = hp + 1
                ctx_tiles[hp] = [psctx.tile([65, 512], F32, tag="ctx",
                                            name=f"ctx{hp}_{i}")
                                 for i in range(2)]
                for t in range(8):
                    ats = []
                    for hh in range(2):
                        lo = 64 * hh
                        sc = pssc.tile([P, 512], F32, tag="sc",
                                       name=f"sc{hp}_{t}_{hh}")
                        nc.tensor.matmul(
                            sc[:],
                            KT[lo:lo + 64, hp, t * P:(t + 1) * P],
                            qT[lo:lo + 64, hp, :],
                            start=True, stop=True,
                        )
                        at = attnp.tile([P, 512], BF16, tag="at")
                        nc.scalar.activation(at[:], sc[:], Exp)
                        ats.append(at)
                    if pend is not None:
                        phh, pt, pats = pend
                        emit_ctx(phh, pt, pats)
                        if pt == 7:
                            emit_norm(phh, ctx_tiles[phh])
                    if nxt < 8:
                        if t == 2:
                            emit_kproj(nxt, 0)
                        elif t == 4:
                            emit_kproj(nxt, 1)
                        elif t == 6:
                            emit_qproj(nxt)
                    else:
                        # keep PE fed while ACT drains: pre-accumulate the
                        # first two out-proj chunks over fj=0..6
                        if t == 2 or t == 4:
                            mi = 0 if t == 2 else 1
                            po = psmm.tile([P, 512], F32, tag="pp",
                                           name=f"ppo0_{mi}")
                            o_chunk_matmuls(po, 0, mi, range(7), True, False)
                            opart.append((po, 0, mi))
                    pend = (hp, t, ats)

            emit_ctx(7, 7, pend[2])
            emit_norm(7, ctx_tiles[7])
            # two more partial chunks from the now-free score banks fill the
            # PE while the hp7 normalization chain drains
            for mi in (2, 3):
                po = pssc.tile([P, 512], F32, tag="sc", name=f"ppo0_{mi}")
                o_chunk_matmuls(po, 0, mi, range(7), True, False)
                opart.append((po, 0, mi))

            # ---- epilogue: finish out projection ----
            for po, half, mi in opart:
                o_chunk_matmuls(po, half, mi, [7], False, True)
                o_chunk_finish(po, half, mi)
            for mi in range(4):
                po = psmm.tile([P, 512], F32, tag="pp", name=f"ppo1_{mi}")
                o_chunk_matmuls(po, 1, mi, range(8), True, True)
                o_chunk_finish(po, 1, mi)

    nc.compile()
    return nc


def _prep_inputs(hidden_states, key_value_states, q_weight, q_bias,
                 kv_weight, kv_bias, out_weight, out_bias):
    f32 = np.float32
    bf16 = ml_dtypes.bfloat16
    hid = np.asarray(hidden_states, f32).reshape(B * LQ, D)
    kv = np.asarray(key_value_states, f32).reshape(B * LK, D)
    scale = f32(1.0 / 8.0)

    # de-interleave kv rows: row e <-> (h=e//128, j=(e%128)//64, d=e%64)
    e = np.arange(2 * D)
    kmask = (e % 128) < 64
    kidx, vidx = e[kmask], e[~kmask]
    kvw = np.asarray(kv_weight, f32)
    kvb = np.asarray(kv_bias, f32)

    shared = {
        "wq_t": np.ascontiguousarray((np.asarray(q_weight, f32) * scale).T.astype(bf16)),
        "wk_t": np.ascontiguousarray(kvw[kidx].T.astype(bf16)),
        "wv_t": np.ascontiguousarray(kvw[vidx].T.astype(bf16)),
        "wo_t": np.ascontiguousarray(np.asarray(out_weight, f32).T.astype(bf16)),
        "qb": np.ascontiguousarray(np.asarray(q_bias, f32) * scale),
        "kb": np.ascontiguousarray(kvb[kidx]),
        "vb": np.ascontiguousarray(kvb[vidx]),
        "ob": np.ascontiguousarray(np.asarray(out_bias, f32)),
    }
    kvT_by_batch = [
        np.ascontiguousarray(kv[b * LK:(b + 1) * LK].T.astype(bf16))
        for b in range(B)
    ]
    in_maps = []
    for c in range(NCORES):
        b = c // 2
        m = dict(shared)
        m["hid_t"] = np.ascontiguousarray(
            hid[c * TQ:(c + 1) * TQ].T.astype(bf16))
        m["kv_t"] = kvT_by_batch[b]
        in_maps.append(m)
    return in_maps


def kernel(hidden_states, key_value_states, q_weight, q_bias,
           kv_weight, kv_bias, out_weight, out_bias, _trace=False):
    if "nc" not in _CACHE:
        _CACHE["nc"] = _build_core_program()
    nc = _CACHE["nc"]
    in_maps = _prep_inputs(hidden_states, key_value_states, q_weight, q_bias,
                           kv_weight, kv_bias, out_weight, out_bias)
    res = bass_utils.run_bass_kernel_spmd(
        nc, in_maps, core_ids=list(range(NCORES)), trace=_trace)
    _CACHE["last_result"] = res
    out = np.concatenate([r["out_s"] for r in res.results], axis=0)
    return out.reshape(B, LQ, D)


# revision 28
# speedup vs baseline: 1.4861x; 1.0137x over previous
"""Trainium2 Bass kernel for nn_BartCrossAttention (B=4, L=1024, D=1024, H=16, HD=64).

Sharding: 8 cores; core c handles query tokens [512c, 512c+512) of batch b=c//2.
Each core recomputes K/V projections for its whole batch (no collectives); the
host slices inputs per core and concatenates outputs.

Design notes:
- All matmul operands bf16 (PE still 1 cycle/row, halves DMA+SBUF traffic;
  measured end-to-end rel err ~6e-3 vs the 2e-2 budget).
- hid/kv transposed on the HOST - no on-device PE transposes at all.
- ctx matmuls are software-pipelined one t-iteration behind S/exp so the PE
  never waits on the ACT engine inside an iteration.
- Softmax normalization is off-PE: ones column in the ctx matmul gives the
  denominator row, reciprocal_approx_fast on DVE, gpsimd partition_broadcast,
  and the PSUM->SBUF ctx eviction does the multiply (normalize-on-evict).
- K/Q projections for pair hp+1 interleave into pair hp's t-loop; during the
  last pair (no projections left) the first two out-projection chunks
  pre-accumulate fj=0..6 so the PE stays fed while ACT drains.
- DMA dispatches are ordered first-needed-first (each dma_start costs ~1us of
  Sync dispatch); ACT exp table is pre-warmed in the prologue.
"""
import sys

for _p in ("/opt/trn_rl_repo",):
    if _p not in sys.path:
        sys.path.insert(0, _p)

import numpy as np
import ml_dtypes

import concourse.bass as bass
import concourse.mybir as mybir
import concourse.tile as tile
from concourse import bacc
import concourse.bass_utils as bass_utils

F32 = mybir.dt.float32
BF16 = mybir.dt.bfloat16

P = 128
D = 1024        # model dim
H = 16          # heads
NCORES = 8
TQ = 512        # query tokens per core
LK = 1024       # kv tokens per batch
B, LQ = 4, 1024

_CACHE = {}


def _build_core_program():
    nc = bacc.Bacc("TRN2", target_bir_lowering=False, debug=False,
                   num_devices=NCORES)

    hid_t = nc.dram_tensor("hid_t", [D, TQ], BF16, kind="ExternalInput")
    kv_t = nc.dram_tensor("kv_t", [D, LK], BF16, kind="ExternalInput")
    wq_t = nc.dram_tensor("wq_t", [D, D], BF16, kind="ExternalInput")
    wk_t = nc.dram_tensor("wk_t", [D, D], BF16, kind="ExternalInput")
    wv_t = nc.dram_tensor("wv_t", [D, D], BF16, kind="ExternalInput")
    wo_t = nc.dram_tensor("wo_t", [D, D], BF16, kind="ExternalInput")
    qb_d = nc.dram_tensor("qb", [D], F32, kind="ExternalInput")
    kb_d = nc.dram_tensor("kb", [D], F32, kind="ExternalInput")
    vb_d = nc.dram_tensor("vb", [D], F32, kind="ExternalInput")
    ob_d = nc.dram_tensor("ob", [D], F32, kind="ExternalInput")
    out_s = nc.dram_tensor("out_s", [TQ, D], F32, kind="ExternalOutput")

    Exp = mybir.ActivationFunctionType.Exp
    add = mybir.AluOpType.add
    mult = mybir.AluOpType.mult

    with tile.TileContext(nc) as tc:
        with (
            tc.tile_pool(name="setup", bufs=1) as setup,
            tc.tile_pool(name="big", bufs=1) as big,
            tc.tile_pool(name="attn", bufs=4) as attnp,
            tc.tile_pool(name="norm", bufs=2) as normp,
            tc.tile_pool(name="outp", bufs=2) as outp,
            tc.tile_pool(name="pssc", bufs=2, space="PSUM") as pssc,
            tc.tile_pool(name="psctx", bufs=4, space="PSUM") as psctx,
            tc.tile_pool(name="psmm", bufs=2, space="PSUM") as psmm,
        ):
            # ---- persistent big tiles ----
            kvT = big.tile([P, 8, LK], BF16, tag="kvT")      # kv^T [1024,1024]
            hidT = big.tile([P, 8, TQ], BF16, tag="hidT")    # hid^T [1024,512]
            wv = big.tile([P, 8, D], BF16, tag="wv")
            wk = big.tile([P, 8, D], BF16, tag="wk")
            wq = big.tile([P, 8, D], BF16, tag="wq")
            wo = big.tile([P, 8, D], BF16, tag="wo")
            KT = big.tile([P, 8, LK], BF16, tag="KT")        # K^T per pair
            qT = big.tile([P, 8, TQ], BF16, tag="qT")        # Q^T per pair
            v65 = big.tile([P, 8, H * 65], BF16, tag="v65")  # V+ones col
            ctxT = big.tile([P, 8, TQ], BF16, tag="ctxT")    # normalized ctx^T

            # ---- DMA dispatch order: first-needed first. The DMA engine
            # round-robins all pending transfers, so later bulk loads are
            # token-gated (1-elem DVE copy into the dst tile = WAR dep) to
            # keep them from stealing bandwidth from the critical prologue.
            # fine-grained first chunks so V-proj starts ~14us; wv half 1 is
            # token-gated below so the early window carries only the 3MB that
            # V half 0 actually consumes
            kv_re = kv_t.ap().rearrange("(dd p) t -> p dd t", p=P)
            wv_re = wv_t.ap().rearrange("(dd p) o -> p dd o", p=P)
            nc.sync.dma_start(kvT[:, :, 0:256], kv_re[:, :, 0:256])
            nc.sync.dma_start(wv[:, 0:4, 0:512], wv_re[:, 0:4, 0:512])
            nc.sync.dma_start(wv[:, 4:8, 0:512], wv_re[:, 4:8, 0:512])
            nc.sync.dma_start(kvT[:, :, 256:512], kv_re[:, :, 256:512])
            nc.sync.dma_start(kvT[:, :, 512:1024], kv_re[:, :, 512:1024])

            # ---- setup: biases, ones, ACT table warm ----
            qb_sb = setup.tile([P, 8], F32, tag="qb")
            nc.sync.dma_start(qb_sb[:], qb_d.ap().rearrange("(o p) -> p o", p=P))
            kb_sb = setup.tile([P, 8], F32, tag="kb")
            nc.sync.dma_start(kb_sb[:], kb_d.ap().rearrange("(o p) -> p o", p=P))
            vbB = setup.tile([P, D], F32, tag="vbB")
            obB = setup.tile([P, D], F32, tag="obB")
            vb_row = setup.tile([1, D], F32, tag="vbrow")
            nc.sync.dma_start(vb_row[:], vb_d.ap()[None, :])
            nc.gpsimd.partition_broadcast(vbB[:], vb_row[:])
            ob_row = setup.tile([1, D], F32, tag="obrow")
            nc.sync.dma_start(ob_row[:], ob_d.ap()[None, :])
            nc.gpsimd.partition_broadcast(obB[:], ob_row[:])

            onesF = setup.tile([P, P], F32, tag="onesF")
            nc.gpsimd.memset(onesF[:], 1.0)
            warm = setup.tile([1, 8], BF16, tag="warm")
            nc.scalar.activation(warm[:], onesF[0:1, 0:8], Exp)

            # ones columns of v65 (col 64 of each head block)
            nc.vector.tensor_copy(
                v65[:].rearrange("p t (h x) -> p t h x", x=65)[:, :, :, 64:65],
                onesF[:].rearrange("p (t h x) -> p t h x", t=8, h=16))

            # ---- V projection (with token-gated weight loads) ----
            v65v = v65[:].rearrange("p t (h x) -> p t h x", x=65)

            def gated_dma(dst_tile, dst_ap, src_ap):
                nc.vector.tensor_copy(dst_tile[0:1, 0, 0:1],
                                      v65[0:1, 0, 0:1])
                nc.sync.dma_start(dst_ap, src_ap)

            for half in range(2):
                for ti in range(8):
                    pp = psmm.tile([P, 512], F32, tag="pp",
                                   name=f"ppv{half}_{ti}")
                    for di in range(8):
                        nc.tensor.matmul(
                            pp[:],
                            kvT[:, di, ti * P:(ti + 1) * P],
                            wv[:, di, half * 512:(half + 1) * 512],
                            start=(di == 0), stop=(di == 7),
                        )
                    nc.vector.tensor_tensor(
                        v65v[:, ti, half * 8:(half + 1) * 8, 0:64], pp[:],
                        vbB[:, half * 512:(half + 1) * 512], add)
                    if half == 0 and ti == 1:
                        # token INSIDE the dst region (col 512), not col 0
                        nc.vector.tensor_copy(wv[0:1, 0, 512:513],
                                              v65[0:1, 0, 0:1])
                        nc.sync.dma_start(wv[:, :, 512:1024],
                                          wv_re[:, :, 512:1024])
                    elif half == 0 and ti == 7:
                        gated_dma(wk, wk[:],
                                  wk_t.ap().rearrange("(dd p) o -> p dd o", p=P))
                    elif half == 1 and ti == 1:
                        gated_dma(hidT, hidT[:],
                                  hid_t.ap().rearrange("(dd p) t -> p dd t", p=P))
                    elif half == 1 and ti == 4:
                        gated_dma(wq, wq[:],
                                  wq_t.ap().rearrange("(dd p) o -> p dd o", p=P))
                    elif half == 1 and ti == 7:
                        gated_dma(wo, wo[:],
                                  wo_t.ap().rearrange("(dd p) o -> p dd o", p=P))

            # ---- K/Q projections (pair 0 now, rest interleaved) ----
            def emit_kproj(hp, nk):
                pp = psmm.tile([P, 512], F32, tag="pp", name=f"ppk{hp}_{nk}")
                for di in range(8):
                    nc.tensor.matmul(
                        pp[:],
                        wk[:, di, hp * P:(hp + 1) * P],
                        kvT[:, di, nk * 512:(nk + 1) * 512],
                        start=(di == 0), stop=(di == 7),
                    )
                nc.vector.tensor_scalar(
                    KT[:, hp, nk * 512:(nk + 1) * 512], pp[:],
                    kb_sb[:, hp:hp + 1], None, add)

            def emit_qproj(hp):
                pq = psmm.tile([P, 512], F32, tag="pp", name=f"ppq{hp}")
                for di in range(8):
                    nc.tensor.matmul(
                        pq[:],
                        wq[:, di, hp * P:(hp + 1) * P],
                        hidT[:, di, :],
                        start=(di == 0), stop=(di == 7),
                    )
                nc.vector.tensor_scalar(qT[:, hp, :], pq[:],
                                        qb_sb[:, hp:hp + 1], None, add)

            emit_kproj(0, 0)
            emit_kproj(0, 1)
            emit_qproj(0)

            # normalization: all off-PE. Copy the sums row out of PSUM, DMA it
            # to partition 0, gpsimd-broadcast the raw sums, then a 64-lane
            # approx reciprocal (18-bit accurate, plenty for well-conditioned
            # positive denominators) and normalize-on-evict.
            def emit_norm(hp, ctx_ps):
                # both heads' chains interleaved to overlap the serial
                # copy -> DMA -> broadcast -> reciprocal -> multiply latency
                srows, r0s, bcs, rcs = [], [], [], []
                for hh in range(2):
                    srow = normp.tile([65, 512], F32, tag="srow",
                                      name=f"srow{hp}_{hh}")
                    nc.vector.tensor_copy(srow[64:65, :], ctx_ps[hh][64:65, :])
                    srows.append(srow)
                for hh in range(2):
                    r0 = normp.tile([1, 512], F32, tag="r0",
                                    name=f"r0_{hp}_{hh}")
                    nc.sync.dma_start(r0[:], srows[hh][64:65, :])
                    r0s.append(r0)
                for hh in range(2):
                    bc = normp.tile([64, 512], F32, tag="bc",
                                    name=f"bc{hp}_{hh}")
                    nc.gpsimd.partition_broadcast(bc[:], r0s[hh][:])
                    bcs.append(bc)
                for hh in range(2):
                    rc = normp.tile([64, 512], F32, tag="rc",
                                    name=f"rc{hp}_{hh}")
                    nc.vector.reciprocal_approx_fast(rc[:], bcs[hh][:])
                    rcs.append(rc)
                nc.vector.tensor_tensor(
                    ctxT[0:64, hp, :], ctx_ps[0][0:64, :], rcs[0][:], mult)
                stg = normp.tile([64, 512], BF16, tag="stg", name=f"stg{hp}")
                nc.vector.tensor_tensor(stg[:], ctx_ps[1][0:64, :],
                                        rcs[1][:], mult)
                nc.sync.dma_start(ctxT[64:128, hp, :], stg[:])

            # out-projection chunk helpers (epilogue + hp7 partials)
            def o_chunk_matmuls(po, half, mi, fjs, start0, stop7):
                for fj in fjs:
                    nc.tensor.matmul(
                        po[:],
                        ctxT[:, fj, mi * P:(mi + 1) * P],
                        wo[:, fj, half * 512:(half + 1) * 512],
                        start=(fj == 0 and start0), stop=(fj == 7 and stop7),
                    )

            def o_chunk_finish(po, half, mi):
                ot = outp.tile([P, 512], F32, tag="ot")
                nc.vector.tensor_tensor(
                    ot[:], po[:], obB[:, half * 512:(half + 1) * 512], add)
                nc.sync.dma_start(
                    out_s.ap().rearrange("(mm p) d -> p mm d", p=P)[
                        :, mi, half * 512:(half + 1) * 512],
                    ot[:])

            # ---- main attention loop (ctx pipelined 1 iter behind) ----
            ctx_tiles = {}
            pend = None  # (hp, t, [at_e, at_o])
            opart = []   # hp7 partial out-proj chunks: (po, half, mi)

            def emit_ctx(hp, t, ats):
                for hh in range(2):
                    h = 2 * hp + hh
                    nc.tensor.matmul(
                        ctx_tiles[hp][hh][:],
                        v65[:, t, h * 65:(h + 1) * 65],
                        ats[hh][:],
                        start=(t == 0), stop=(t == 7),
                    )

            for hp in range(8):
                nxt = hp + 1
                ctx_tiles[hp] = [psctx.tile([65, 512], F32, tag="ctx",
                                            name=f"ctx{hp}_{i}")
                                 for i in range(2)]
                for t in range(8):
                    ats = []
                    for hh in range(2):
                        lo = 64 * hh
                        sc = pssc.tile([P, 512], F32, tag="sc",
                                       name=f"sc{hp}_{t}_{hh}")
                        nc.tensor.matmul(
                            sc[:],
                            KT[lo:lo + 64, hp, t * P:(t + 1) * P],
                            qT[lo:lo + 64, hp, :],
                            start=True, stop=True,
                        )
                        at = attnp.tile([P, 512], BF16, tag="at")
                        nc.scalar.activation(at[:], sc[:], Exp)
                        ats.append(at)
                    if pend is not None:
                        phh, pt, pats = pend
                        emit_ctx(phh, pt, pats)
                        if pt == 7:
                            emit_norm(phh, ctx_tiles[phh])
                    if nxt < 8:
                        if t == 2:
                            emit_kproj(nxt, 0)
                        elif t == 4:
                            emit_kproj(nxt, 1)
                        elif t == 6:
                            emit_qproj(nxt)
                    else:
                        # keep PE fed while ACT drains: pre-accumulate the
                        # first two out-proj chunks over fj=0..6
                        if t == 2 or t == 4:
                            mi = 0 if t == 2 else 1
                            po = psmm.tile([P, 512], F32, tag="pp",
                                           name=f"ppo0_{mi}")
                            o_chunk_matmuls(po, 0, mi, range(7), True, False)
                            opart.append((po, 0, mi))
                    pend = (hp, t, ats)

            emit_ctx(7, 7, pend[2])
            emit_norm(7, ctx_tiles[7])
            # two more partial chunks from the now-free score banks fill the
            # PE while the hp7 normalization chain drains
            for mi in (2, 3):
                po = pssc.tile([P, 512], F32, tag="sc", name=f"ppo0_{mi}")
                o_chunk_matmuls(po, 0, mi, range(7), True, False)
                opart.append((po, 0, mi))

            # ---- epilogue: finish out projection ----
            for po, half, mi in opart:
                o_chunk_matmuls(po, half, mi, [7], False, True)
                o_chunk_finish(po, half, mi)
            for mi in range(4):
                po = psmm.tile([P, 512], F32, tag="pp", name=f"ppo1_{mi}")
                o_chunk_matmuls(po, 1, mi, range(8), True, True)
                o_chunk_finish(po, 1, mi)

    nc.compile()
    return nc


def _prep_inputs(hidden_states, key_value_states, q_weight, q_bias,
                 kv_weight, kv_bias, out_weight, out_bias):
    f32 = np.float32
    bf16 = ml_dtypes.bfloat16
    hid = np.asarray(hidden_states, f32).reshape(B * LQ, D)
    kv = np.asarray(key_value_states, f32).reshape(B * LK, D)
    scale = f32(1.0 / 8.0)

    # de-interleave kv rows: row e <-> (h=e//128, j=(e%128)//64, d=e%64)
    e = np.arange(2 * D)
    kmask = (e % 128) < 64
    kidx, vidx = e[kmask], e[~kmask]
    kvw = np.asarray(kv_weight, f32)
    kvb = np.asarray(kv_bias, f32)

    shared = {
        "wq_t": np.ascontiguousarray((np.asarray(q_weight, f32) * scale).T.astype(bf16)),
        "wk_t": np.ascontiguousarray(kvw[kidx].T.astype(bf16)),
        "wv_t": np.ascontiguousarray(kvw[vidx].T.astype(bf16)),
        "wo_t": np.ascontiguousarray(np.asarray(out_weight, f32).T.astype(bf16)),
        "qb": np.ascontiguousarray(np.asarray(q_bias, f32) * scale),
        "kb": np.ascontiguousarray(kvb[kidx]),
        "vb": np.ascontiguousarray(kvb[vidx]),
        "ob": np.ascontiguousarray(np.asarray(out_bias, f32)),
    }
    kvT_by_batch = [
        np.ascontiguousarray(kv[b * LK:(b + 1) * LK].T.astype(bf16))
        for b in range(B)
    ]
    in_maps = []
    for c in range(NCORES):
        b = c // 2
        m = dict(shared)
        m["hid_t"] = np.ascontiguousarray(
            hid[c * TQ:(c + 1) * TQ].T.astype(bf16))
        m["kv_t"] = kvT_by_batch[b]
        in_maps.append(m)
    return in_maps


def kernel(hidden_states, key_value_states, q_weight, q_bias,
           kv_weight, kv_bias, out_weight, out_bias, _trace=False):
    if "nc" not in _CACHE:
        _CACHE["nc"] = _build_core_program()
    nc = _CACHE["nc"]
    in_maps = _prep_inputs(hidden_states, key_value_states, q_weight, q_bias,
                           kv_weight, kv_bias, out_weight, out_bias)
    res = bass_utils.run_bass_kernel_spmd(
        nc, in_maps, core_ids=list(range(NCORES)), trace=_trace)
    _CACHE["last_result"] = res
    out = np.concatenate([r["out_s"] for r in res.results], axis=0)
    return out.reshape(B, LQ, D)


# revision 31
# speedup vs baseline: 1.4881x; 1.0013x over previous
"""Trainium2 Bass kernel for nn_BartCrossAttention (B=4, L=1024, D=1024, H=16, HD=64).

Sharding: 8 cores; core c handles query tokens [512c, 512c+512) of batch b=c//2.
Each core recomputes K/V projections for its whole batch (no collectives); the
host slices inputs per core and concatenates outputs.

Design notes:
- All matmul operands bf16 (PE still 1 cycle/row, halves DMA+SBUF traffic;
  measured end-to-end rel err ~6e-3 vs the 2e-2 budget).
- hid/kv transposed on the HOST - no on-device PE transposes at all.
- ctx matmuls are software-pipelined one t-iteration behind S/exp so the PE
  never waits on the ACT engine inside an iteration.
- Softmax normalization is off-PE: ones column in the ctx matmul gives the
  denominator row, reciprocal_approx_fast on DVE, gpsimd partition_broadcast,
  and the PSUM->SBUF ctx eviction does the multiply (normalize-on-evict).
- K/Q projections for pair hp+1 interleave into pair hp's t-loop; during the
  last pair (no projections left) the first two out-projection chunks
  pre-accumulate fj=0..6 so the PE stays fed while ACT drains.
- DMA dispatches are ordered first-needed-first (each dma_start costs ~1us of
  Sync dispatch); ACT exp table is pre-warmed in the prologue.
"""
import sys

for _p in ("/opt/trn_rl_repo",):
    if _p not in sys.path:
        sys.path.insert(0, _p)

import numpy as np
import ml_dtypes

import concourse.bass as bass
import concourse.mybir as mybir
import concourse.tile as tile
from concourse import bacc
import concourse.bass_utils as bass_utils

F32 = mybir.dt.float32
BF16 = mybir.dt.bfloat16

P = 128
D = 1024        # model dim
H = 16          # heads
NCORES = 8
TQ = 512        # query tokens per core
LK = 1024       # kv tokens per batch
B, LQ = 4, 1024

_CACHE = {}


def _build_core_program():
    nc = bacc.Bacc("TRN2", target_bir_lowering=False, debug=False,
                   num_devices=NCORES)

    hid_t = nc.dram_tensor("hid_t", [D, TQ], BF16, kind="ExternalInput")
    kv_t = nc.dram_tensor("kv_t", [D, LK], BF16, kind="ExternalInput")
    wq_t = nc.dram_tensor("wq_t", [D, D], BF16, kind="ExternalInput")
    wk_t = nc.dram_tensor("wk_t", [D, D], BF16, kind="ExternalInput")
    wv_t = nc.dram_tensor("wv_t", [D, D], BF16, kind="ExternalInput")
    wo_t = nc.dram_tensor("wo_t", [D, D], BF16, kind="ExternalInput")
    qb_d = nc.dram_tensor("qb", [D], F32, kind="ExternalInput")
    kb_d = nc.dram_tensor("kb", [D], F32, kind="ExternalInput")
    vb_d = nc.dram_tensor("vb", [D], F32, kind="ExternalInput")
    ob_d = nc.dram_tensor("ob", [D], F32, kind="ExternalInput")
    out_s = nc.dram_tensor("out_s", [TQ, D], F32, kind="ExternalOutput")

    Exp = mybir.ActivationFunctionType.Exp
    add = mybir.AluOpType.add
    mult = mybir.AluOpType.mult

    with tile.TileContext(nc) as tc:
        with (
            tc.tile_pool(name="setup", bufs=1) as setup,
            tc.tile_pool(name="big", bufs=1) as big,
            tc.tile_pool(name="attn", bufs=6) as attnp,
            tc.tile_pool(name="norm", bufs=2) as normp,
            tc.tile_pool(name="outp", bufs=2) as outp,
            tc.tile_pool(name="pssc", bufs=2, space="PSUM") as pssc,
            tc.tile_pool(name="psctx", bufs=4, space="PSUM") as psctx,
            tc.tile_pool(name="psmm", bufs=2, space="PSUM") as psmm,
        ):
            # ---- persistent big tiles ----
            kvT = big.tile([P, 8, LK], BF16, tag="kvT")      # kv^T [1024,1024]
            hidT = big.tile([P, 8, TQ], BF16, tag="hidT")    # hid^T [1024,512]
            wv = big.tile([P, 8, D], BF16, tag="wv")
            wk = big.tile([P, 8, D], BF16, tag="wk")
            wq = big.tile([P, 8, D], BF16, tag="wq")
            wo = big.tile([P, 8, D], BF16, tag="wo")
            KT = big.tile([P, 8, LK], BF16, tag="KT")        # K^T per pair
            qT = big.tile([P, 8, TQ], BF16, tag="qT")        # Q^T per pair
            v65 = big.tile([P, 8, H * 65], BF16, tag="v65")  # V+ones col
            ctxT = big.tile([P, 8, TQ], BF16, tag="ctxT")    # normalized ctx^T

            # ---- DMA dispatch order: first-needed first. The DMA engine
            # round-robins all pending transfers, so later bulk loads are
            # token-gated (1-elem DVE copy into the dst tile = WAR dep) to
            # keep them from stealing bandwidth from the critical prologue.
            # fine-grained first chunks so V-proj starts ~14us; wv half 1 is
            # token-gated below so the early window carries only the 3MB that
            # V half 0 actually consumes
            kv_re = kv_t.ap().rearrange("(dd p) t -> p dd t", p=P)
            wv_re = wv_t.ap().rearrange("(dd p) o -> p dd o", p=P)
            nc.sync.dma_start(kvT[:, :, 0:256], kv_re[:, :, 0:256])
            nc.sync.dma_start(wv[:, 0:4, 0:512], wv_re[:, 0:4, 0:512])
            nc.sync.dma_start(wv[:, 4:8, 0:512], wv_re[:, 4:8, 0:512])
            nc.sync.dma_start(kvT[:, :, 256:512], kv_re[:, :, 256:512])
            nc.sync.dma_start(kvT[:, :, 512:1024], kv_re[:, :, 512:1024])

            # ---- setup: biases, ones, ACT table warm ----
            # (only vb is needed during V-proj; qb/kb/ob dispatches are
            # deferred past the V loop to keep the early DMA window clean)
            qb_sb = setup.tile([P, 8], F32, tag="qb")
            kb_sb = setup.tile([P, 8], F32, tag="kb")
            vbB = setup.tile([P, D], F32, tag="vbB")
            obB = setup.tile([P, D], F32, tag="obB")
            vb_row = setup.tile([1, D], F32, tag="vbrow")
            nc.sync.dma_start(vb_row[:], vb_d.ap()[None, :])
            nc.gpsimd.partition_broadcast(vbB[:], vb_row[:])
            ob_row = setup.tile([1, D], F32, tag="obrow")

            onesF = setup.tile([P, P], F32, tag="onesF")
            nc.gpsimd.memset(onesF[:], 1.0)
            warm = setup.tile([1, 8], BF16, tag="warm")
            nc.scalar.activation(warm[:], onesF[0:1, 0:8], Exp)

            # ones columns of v65 (col 64 of each head block)
            nc.vector.tensor_copy(
                v65[:].rearrange("p t (h x) -> p t h x", x=65)[:, :, :, 64:65],
                onesF[:].rearrange("p (t h x) -> p t h x", t=8, h=16))

            # ---- V projection (with token-gated weight loads) ----
            v65v = v65[:].rearrange("p t (h x) -> p t h x", x=65)

            def gated_dma(dst_tile, dst_ap, src_ap):
                nc.vector.tensor_copy(dst_tile[0:1, 0, 0:1],
                                      v65[0:1, 0, 0:1])
                nc.sync.dma_start(dst_ap, src_ap)

            for half in range(2):
                for ti in range(8):
                    pp = psmm.tile([P, 512], F32, tag="pp",
                                   name=f"ppv{half}_{ti}")
                    for di in range(8):
                        nc.tensor.matmul(
                            pp[:],
                            kvT[:, di, ti * P:(ti + 1) * P],
                            wv[:, di, half * 512:(half + 1) * 512],
                            start=(di == 0), stop=(di == 7),
                        )
                    nc.vector.tensor_tensor(
                        v65v[:, ti, half * 8:(half + 1) * 8, 0:64], pp[:],
                        vbB[:, half * 512:(half + 1) * 512], add)
                    if half == 0 and ti == 1:
                        # token INSIDE the dst region (col 512), not col 0
                        nc.vector.tensor_copy(wv[0:1, 0, 512:513],
                                              v65[0:1, 0, 0:1])
                        nc.sync.dma_start(wv[:, :, 512:1024],
                                          wv_re[:, :, 512:1024])
                    elif half == 0 and ti == 7:
                        gated_dma(wk, wk[:],
                                  wk_t.ap().rearrange("(dd p) o -> p dd o", p=P))
                    elif half == 1 and ti == 1:
                        gated_dma(hidT, hidT[:],
                                  hid_t.ap().rearrange("(dd p) t -> p dd t", p=P))
                    elif half == 1 and ti == 4:
                        gated_dma(wq, wq[:],
                                  wq_t.ap().rearrange("(dd p) o -> p dd o", p=P))
                    elif half == 1 and ti == 7:
                        gated_dma(wo, wo[:],
                                  wo_t.ap().rearrange("(dd p) o -> p dd o", p=P))

            # ---- K/Q projections (pair 0 now, rest interleaved) ----
            def emit_kproj(hp, nk):
                pp = psmm.tile([P, 512], F32, tag="pp", name=f"ppk{hp}_{nk}")
                for di in range(8):
                    nc.tensor.matmul(
                        pp[:],
                        wk[:, di, hp * P:(hp + 1) * P],
                        kvT[:, di, nk * 512:(nk + 1) * 512],
                        start=(di == 0), stop=(di == 7),
                    )
                nc.vector.tensor_scalar(
                    KT[:, hp, nk * 512:(nk + 1) * 512], pp[:],
                    kb_sb[:, hp:hp + 1], None, add)

            def emit_qproj(hp):
                pq = psmm.tile([P, 512], F32, tag="pp", name=f"ppq{hp}")
                for di in range(8):
                    nc.tensor.matmul(
                        pq[:],
                        wq[:, di, hp * P:(hp + 1) * P],
                        hidT[:, di, :],
                        start=(di == 0), stop=(di == 7),
                    )
                nc.vector.tensor_scalar(qT[:, hp, :], pq[:],
                                        qb_sb[:, hp:hp + 1], None, add)

            # deferred small-bias loads (needed from K0/Q0 and the epilogue)
            nc.sync.dma_start(qb_sb[:], qb_d.ap().rearrange("(o p) -> p o", p=P))
            nc.sync.dma_start(kb_sb[:], kb_d.ap().rearrange("(o p) -> p o", p=P))
            nc.sync.dma_start(ob_row[:], ob_d.ap()[None, :])
            nc.gpsimd.partition_broadcast(obB[:], ob_row[:])

            emit_kproj(0, 0)
            emit_kproj(0, 1)
            emit_qproj(0)

            # normalization: all off-PE. Copy the sums row out of PSUM, DMA it
            # to partition 0, gpsimd-broadcast the raw sums, then a 64-lane
            # approx reciprocal (18-bit accurate, plenty for well-conditioned
            # positive denominators) and normalize-on-evict.
            def emit_norm(hp, ctx_ps):
                # both heads' chains interleaved to overlap the serial
                # copy -> DMA -> broadcast -> reciprocal -> multiply latency
                srows, r0s, bcs, rcs = [], [], [], []
                for hh in range(2):
                    srow = normp.tile([65, 512], F32, tag="srow",
                                      name=f"srow{hp}_{hh}")
                    nc.vector.tensor_copy(srow[64:65, :], ctx_ps[hh][64:65, :])
                    srows.append(srow)
                for hh in range(2):
                    r0 = normp.tile([1, 512], F32, tag="r0",
                                    name=f"r0_{hp}_{hh}")
                    nc.sync.dma_start(r0[:], srows[hh][64:65, :])
                    r0s.append(r0)
                for hh in range(2):
                    bc = normp.tile([64, 512], F32, tag="bc",
                                    name=f"bc{hp}_{hh}")
                    nc.gpsimd.partition_broadcast(bc[:], r0s[hh][:])
                    bcs.append(bc)
                for hh in range(2):
                    rc = normp.tile([64, 512], F32, tag="rc",
                                    name=f"rc{hp}_{hh}")
                    nc.vector.reciprocal_approx_fast(rc[:], bcs[hh][:])
                    rcs.append(rc)
                nc.vector.tensor_tensor(
                    ctxT[0:64, hp, :], ctx_ps[0][0:64, :], rcs[0][:], mult)
                stg = normp.tile([64, 512], BF16, tag="stg", name=f"stg{hp}")
                nc.vector.tensor_tensor(stg[:], ctx_ps[1][0:64, :],
                                        rcs[1][:], mult)
                nc.sync.dma_start(ctxT[64:128, hp, :], stg[:])

            # out-projection chunk helpers (epilogue + hp7 partials)
            def o_chunk_matmuls(po, half, mi, fjs, start0, stop7):
                for fj in fjs:
                    nc.tensor.matmul(
                        po[:],
                        ctxT[:, fj, mi * P:(mi + 1) * P],
                        wo[:, fj, half * 512:(half + 1) * 512],
                        start=(fj == 0 and start0), stop=(fj == 7 and stop7),
                    )

            def o_chunk_finish(po, half, mi):
                ot = outp.tile([P, 512], F32, tag="ot")
                nc.vector.tensor_tensor(
                    ot[:], po[:], obB[:, half * 512:(half + 1) * 512], add)
                nc.sync.dma_start(
                    out_s.ap().rearrange("(mm p) d -> p mm d", p=P)[
                        :, mi, half * 512:(half + 1) * 512],
                    ot[:])

            # ---- main attention loop (ctx pipelined 1 iter behind) ----
            ctx_tiles = {}
            pend = None  # (hp, t, [at_e, at_o])
            opart = []   # hp7 partial out-proj chunks: (po, half, mi)

            def emit_ctx(hp, t, ats):
                for hh in range(2):
                    h = 2 * hp + hh
                    nc.tensor.matmul(
                        ctx_tiles[hp][hh][:],
                        v65[:, t, h * 65:(h + 1) * 65],
                        ats[hh][:],
                        start=(t == 0), stop=(t == 7),
                    )

            for hp in range(8):
                nxt = hp + 1
                ctx_tiles[hp] = [psctx.tile([65, 512], F32, tag="ctx",
                                            name=f"ctx{hp}_{i}")
                                 for i in range(2)]
                for t in range(8):
                    ats = []
                    for hh in range(2):
                        lo = 64 * hh
                        sc = pssc.tile([P, 512], F32, tag="sc",
                                       name=f"sc{hp}_{t}_{hh}")
                        nc.tensor.matmul(
                            sc[:],
                            KT[lo:lo + 64, hp, t * P:(t + 1) * P],
                            qT[lo:lo + 64, hp, :],
                            start=True, stop=True,
                        )
                        at = attnp.tile([P, 512], BF16, tag="at")
                        nc.scalar.activation(at[:], sc[:], Exp)
                        ats.append(at)
                    if pend is not None:
                        phh, pt, pats = pend
                        emit_ctx(phh, pt, pats)
                        if pt == 7:
                            emit_norm(phh, ctx_tiles[phh])
                    if nxt < 8:
                        if t == 2:
                            emit_kproj(nxt, 0)
                        elif t == 4:
                            emit_kproj(nxt, 1)
                        elif t == 6:
                            emit_qproj(nxt)
                    else:
                        # keep PE fed while ACT drains: pre-accumulate the
                        # first two out-proj chunks over fj=0..6
                        if t == 2 or t == 4:
                            mi = 0 if t == 2 else 1
                            po = psmm.tile([P, 512], F32, tag="pp",
                                           name=f"ppo0_{mi}")
                            o_chunk_matmuls(po, 0, mi, range(7), True, False)
                            opart.append((po, 0, mi))
                    pend = (hp, t, ats)

            emit_ctx(7, 7, pend[2])
            emit_norm(7, ctx_tiles[7])
            # two more partial chunks from the now-free score banks fill the
            # PE while the hp7 normalization chain drains
            for mi in (2, 3):
                po = pssc.tile([P, 512], F32, tag="sc", name=f"ppo0_{mi}")
                o_chunk_matmuls(po, 0, mi, range(7), True, False)
                opart.append((po, 0, mi))

            # ---- epilogue: finish out projection ----
            for po, half, mi in opart:
                o_chunk_matmuls(po, half, mi, [7], False, True)
                o_chunk_finish(po, half, mi)
            for mi in range(4):
                po = psmm.tile([P, 512], F32, tag="pp", name=f"ppo1_{mi}")
                o_chunk_matmuls(po, 1, mi, range(8), True, True)
                o_chunk_finish(po, 1, mi)

    nc.compile()
    return nc


def _prep_inputs(hidden_states, key_value_states, q_weight, q_bias,
                 kv_weight, kv_bias, out_weight, out_bias):
    f32 = np.float32
    bf16 = ml_dtypes.bfloat16
    hid = np.asarray(hidden_states, f32).reshape(B * LQ, D)
    kv = np.asarray(key_value_states, f32).reshape(B * LK, D)
    scale = f32(1.0 / 8.0)

    # de-interleave kv rows: row e <-> (h=e//128, j=(e%128)//64, d=e%64)
    e = np.arange(2 * D)
    kmask = (e % 128) < 64
    kidx, vidx = e[kmask], e[~kmask]
    kvw = np.asarray(kv_weight, f32)
    kvb = np.asarray(kv_bias, f32)

    shared = {
        "wq_t": np.ascontiguousarray((np.asarray(q_weight, f32) * scale).T.astype(bf16)),
        "wk_t": np.ascontiguousarray(kvw[kidx].T.astype(bf16)),
        "wv_t": np.ascontiguousarray(kvw[vidx].T.astype(bf16)),
        "wo_t": np.ascontiguousarray(np.asarray(out_weight, f32).T.astype(bf16)),
        "qb": np.ascontiguousarray(np.asarray(q_bias, f32) * scale),
        "kb": np.ascontiguousarray(kvb[kidx]),
        "vb": np.ascontiguousarray(kvb[vidx]),
        "ob": np.ascontiguousarray(np.asarray(out_bias, f32)),
    }
    kvT_by_batch = [
        np.ascontiguousarray(kv[b * LK:(b + 1) * LK].T.astype(bf16))
        for b in range(B)
    ]
    in_maps = []
    for c in range(NCORES):
        b = c // 2
        m = dict(shared)
        m["hid_t"] = np.ascontiguousarray(
            hid[c * TQ:(c + 1) * TQ].T.astype(bf16))
        m["kv_t"] = kvT_by_batch[b]
        in_maps.append(m)
    return in_maps


def kernel(hidden_states, key_value_states, q_weight, q_bias,
           kv_weight, kv_bias, out_weight, out_bias, _trace=False):
    if "nc" not in _CACHE:
        _CACHE["nc"] = _build_core_program()
    nc = _CACHE["nc"]
    in_maps = _prep_inputs(hidden_states, key_value_states, q_weight, q_bias,
                           kv_weight, kv_bias, out_weight, out_bias)
    res = bass_utils.run_bass_kernel_spmd(
        nc, in_maps, core_ids=list(range(NCORES)), trace=_trace)
    _CACHE["last_result"] = res
    out = np.concatenate([r["out_s"] for r in res.results], axis=0)
    return out.reshape(B, LQ, D)
